# revision 1
# baseline (speedup 1.0000x reference)
"""GAT 2-layer kernel for 8 Trainium2 NeuronCores.

Strategy (dst-sharded edge partitioning):
  - Nodes and their in-edges are sharded by dst across 8 cores (12500 nodes each).
  - Self-loops are appended as regular edges; edges sorted by dst.
  - Per core, dsts are processed in 98 windows of 128. Each window's edges are
    grouped into 4 src-banks (int16-indexable 32768-row overlapping banks of the
    node table) and chunk-padded to a fixed count of 128-edge chunks.
  - A packed per-node table  [xh (128 f16) | a_src (4 f16) | pad]  (512B rows)
    is computed on-device (dense matmuls); per-edge rows are fetched with the
    gpsimd dma_gather custom op (int16 indices, bank-relative).
  - Attention: e = leakyrelu(a_src[src] + a_dst[dst]); softmax without
    max-subtraction (scale-invariant; exponents are small); normalization is
    applied AFTER aggregation:  out[d] = (sum_e ex_e * xh[src_e]) / (sum_e ex_e),
    which removes the second edge pass entirely.
  - Scatter-add inside a window is a one-hot matmul: PSUM[d, :] accumulates
    onehot.T @ [ex | ex*xh_src] over the window's chunks.
  - a_dst expansion to edges reuses the transposed one-hot (PE transpose) as
    adexp = onehotT.T @ a_dst_window.
  - Between layers: AllGather of the (transposed, fp16) h shards; layer 2 is
    identical machinery with head-mean output.
"""

import numpy as np

# ---------------------------------------------------------------- constants
N = 100000
E_IN = 1600000
CORES = 8
M = N // CORES              # 12500 nodes per core
P = 128
WPC = (M + P - 1) // P      # 98 windows per core
H, C = 4, 32                # heads x channels (both layers)
F = 128                     # feature width (= H*C)
ROW = 256                   # f16 elements per table row (512B): [xh 128 | a_src 4 | pad]
BANKS = 4
BANK_ROWS = 32768
BANK_BASES = [0, 22411, 44822, 67232]
PAGE = 8                    # chunks per dma_gather call (1024 idx = ucode ring limit)
NEG_SLOPE = 0.2
EPS = 1e-30


def _host_prep(edge_index):
    """Partition/sort/bank/pad the edge structure. Returns per-core index data.

    All arrays returned are per-core lists:
      idxw      [128, NCALLS*64] int16  -- wrapped bank-relative gather indices
      dstrel    [128, NCHUNKS]   f16    -- dst-relative-to-window per lane (-1 = pad)
    plus the global constants (K chunks per window-bank, NCALLS, NCHUNKS).
    """
    src = np.concatenate([edge_index[0], np.arange(N, dtype=np.int64)]).astype(np.int64)
    dst = np.concatenate([edge_index[1], np.arange(N, dtype=np.int64)]).astype(np.int64)

    bases = np.asarray(BANK_BASES, np.int64)
    cores = []
    maxK = 0
    for m in range(CORES):
        sel = (dst // M) == m
        s_m = src[sel]
        dloc = (dst[sel] - m * M).astype(np.int64)
        win = dloc // P
        order = np.argsort(win, kind="stable")
        s_m, dloc, win = s_m[order], dloc[order], win[order]
        # bank eligibility: hi = last base <= s (always valid); lo = hi-1 if also valid
        hi_b = np.searchsorted(bases, s_m, side="right") - 1
        lo_ok = (hi_b > 0) & (s_m < bases[np.maximum(hi_b - 1, 0)] + BANK_ROWS)
        lo_b = np.where(lo_ok, hi_b - 1, hi_b)
        bank = np.empty(len(s_m), np.int8)
        wstarts = np.searchsorted(win, np.arange(WPC + 1))
        for w in range(WPC):
            a, z = wstarts[w], wstarts[w + 1]
            nb = z - a
            T = -(-nb // BANKS)
            cnt = np.bincount(hi_b[a:z][~lo_ok[a:z]], minlength=BANKS).astype(np.int64)
            bw = hi_b[a:z].copy()
            rigid = ~lo_ok[a:z]
            bw[rigid] = hi_b[a:z][rigid]
            for pnr in range(BANKS - 1):           # pairs (p, p+1), left to right
                fm = lo_ok[a:z] & (lo_b[a:z] == pnr)
                f = int(fm.sum())
                give = min(f, max(0, T - int(cnt[pnr])))
                idxs = np.flatnonzero(fm)
                bw[idxs[:give]] = pnr
                bw[idxs[give:]] = pnr + 1
                cnt[pnr] += give
                cnt[pnr + 1] += f - give
            bank[a:z] = bw
            maxK = max(maxK, int(-(-cnt.max() // P)))
        cores.append((s_m, dloc, win, bank, wstarts))

    K = max(5, maxK)             # chunks per (window, bank)
    CPB = WPC * K                # chunks per bank per layer
    NCALLB = (CPB + PAGE - 1) // PAGE   # calls per bank
    NCHUNKS = WPC * BANKS * K

    out = []
    for m in range(CORES):
        s_m, dloc, win, bank, wstarts = cores[m]
        # flat per-bank chunk streams
        idx_flat = np.zeros((BANKS, CPB * P), np.int16)     # pad idx = 0
        rel_flat = np.full((BANKS, CPB * P), -1.0, np.float32)
        for w in range(WPC):
            a, z = wstarts[w], wstarts[w + 1]
            bw = bank[a:z]
            for b in range(BANKS):
                mask = bw == b
                sl = (s_m[a:z][mask] - BANK_BASES[b]).astype(np.int16)
                rl = (dloc[a:z][mask] - w * P).astype(np.float32)
                base = (w * K) * P
                idx_flat[b, base:base + len(sl)] = sl
                rel_flat[b, base:base + len(rl)] = rl
        # wrap indices per call: call j of bank b covers idx_flat[b, j*1024:(j+1)*1024]
        NIDX = PAGE * P
        S = NIDX // 16
        idxw = np.zeros((P, BANKS * NCALLB * S), np.int16)
        for b in range(BANKS):
            for j in range(NCALLB):
                seg = np.zeros(NIDX, np.int16)
                have = idx_flat[b, j * NIDX:(j + 1) * NIDX]
                seg[:len(have)] = have
                w16 = seg.reshape(S, 16).T                  # [16, S]
                col0 = (b * NCALLB + j) * S
                idxw[:, col0:col0 + S] = np.tile(w16, (CORES, 1))
        # dstrel per chunk, lane-major: [128, NCHUNKS] (chunk order: w-major, bank, k)
        dstrel = np.full((P, NCHUNKS), -1.0, np.float32)
        for w in range(WPC):
            for b in range(BANKS):
                for k in range(K):
                    cid = (w * BANKS + b) * K + k
                    seg = rel_flat[b, (w * K + k) * P:(w * K + k + 1) * P]
                    dstrel[:, cid] = seg
        out.append({"idxw": idxw, "dstrel": dstrel})
    consts = {"K": K, "CPB": CPB, "NCALLB": NCALLB, "NCHUNKS": NCHUNKS,
              "NIDX": PAGE * P, "S": (PAGE * P) // 16}
    return out, consts


def _cat_mats(W, att_src, att_dst):
    """[F_in, F] weight plus block-diag attention columns -> [F_in, 136] f32."""
    F_in = W.shape[0]
    A_src = np.zeros((F, H), np.float32)
    A_dst = np.zeros((F, H), np.float32)
    for h in range(H):
        A_src[h * C:(h + 1) * C, h] = att_src[h]
        A_dst[h * C:(h + 1) * C, h] = att_dst[h]
    return np.concatenate([W.astype(np.float32) @ np.eye(F, dtype=np.float32),
                           W.astype(np.float32) @ A_src,
                           W.astype(np.float32) @ A_dst], axis=1)  # [F_in, 136]


def _emulate_core(m, xh_asrc, adst, perm_data, consts):
    """Numpy replica of the on-device per-core edge phase (one layer).
    xh_asrc: [N, 132] f32-ish (xh | a_src) already in table precision.
    adst:    [N, 4]
    Returns unnormalized [M, 132] psum (ex-sums in cols 0:4, msg in 4:132)."""
    K = consts["K"]
    idxw, dstrel = perm_data["idxw"], perm_data["dstrel"]
    S, NIDX, NCALLB = consts["S"], consts["NIDX"], consts["NCALLB"]
    psum = np.zeros((M + (P * WPC - M), 132), np.float64)  # padded rows
    # reconstruct flat idx per bank from wraps
    for b in range(BANKS):
        for j in range(NCALLB):
            col0 = (b * NCALLB + j) * S
            w16 = idxw[:16, col0:col0 + S]                 # [16, S]
            seg = w16.T.reshape(-1)                        # unwrapped [1024]
            for pg in range(PAGE):
                cglob = j * PAGE + pg                      # chunk within bank stream
                if cglob >= consts["CPB"]:
                    break
                w, k = cglob // K, cglob % K
                cid = (w * BANKS + b) * K + k
                lanes = seg[pg * P:(pg + 1) * P].astype(np.int64) + BANK_BASES[b]
                rows = xh_asrc[lanes]                      # [128, 132]
                rel = dstrel[:, cid].astype(np.float32)    # [128]
                valid = rel >= 0
                d = np.where(valid, rel, 0).astype(np.int64) + w * P
                e = rows[:, 128:132].astype(np.float32) + adst[m * M + d]  # [128,4]
                e = np.where(e >= 0, e, NEG_SLOPE * e)
                ex = np.exp(e).astype(np.float16).astype(np.float32)
                msg = rows[:, :128].astype(np.float32) * np.repeat(ex, C, axis=1)
                contrib = np.concatenate([ex, msg], axis=1) * valid[:, None]
                np.add.at(psum, d, contrib.astype(np.float64))
    return psum[:M]


def emulate(inputs, perm, consts):
    """Full numpy emulation of the device algorithm (all cores), for testing."""
    x = np.asarray(inputs["x"], np.float32)
    cat1 = _cat_mats(np.asarray(inputs["W1"], np.float32),
                     np.asarray(inputs["att_src1"], np.float32),
                     np.asarray(inputs["att_dst1"], np.float32))
    t1 = (x @ cat1).astype(np.float16)                     # [N, 136]
    h_parts = []
    for m in range(CORES):
        ps = _emulate_core(m, t1[:, :132].astype(np.float32),
                           t1[:, 132:136].astype(np.float32), perm[m], consts)
        s = ps[:, 0:4] + EPS
        unn = ps[:, 4:132]
        hm = (unn / np.repeat(s, C, axis=1)).astype(np.float32) + np.asarray(inputs["b1"], np.float32)
        hm = np.where(hm > 0, hm, np.expm1(hm))            # elu
        h_parts.append(hm.astype(np.float16))
    h = np.concatenate(h_parts, axis=0).astype(np.float32) # [N, 128] (fp16 quantized)
    cat2 = _cat_mats(np.asarray(inputs["W2"], np.float32),
                     np.asarray(inputs["att_src2"], np.float32),
                     np.asarray(inputs["att_dst2"], np.float32))
    t2 = (h @ cat2).astype(np.float16)
    outs = []
    for m in range(CORES):
        ps = _emulate_core(m, t2[:, :132].astype(np.float32),
                           t2[:, 132:136].astype(np.float32), perm[m], consts)
        s = ps[:, 0:4] + EPS
        unn = ps[:, 4:132]
        o = (unn / np.repeat(s, C, axis=1)).astype(np.float32)
        o = o.reshape(M, H, C).mean(axis=1) + np.asarray(inputs["b2"], np.float32)
        outs.append(o.astype(np.float32))
    return np.concatenate(outs, axis=0)



# ======================================================================
# device program (Bass/Tile)
# ======================================================================
import concourse.bacc as bacc
import concourse.bass as bass
import concourse.mybir as mybir
import concourse.tile as tile
from concourse.tile import ScopedClock
from concourse.masks import make_identity
from concourse.bass_utils import run_bass_kernel_spmd

F16 = mybir.dt.float16
F32 = mybir.dt.float32
I16 = mybir.dt.int16
NTILES = (N + P - 1) // P          # 782 node tiles in phase 0
AF = mybir.ActivationFunctionType
ALU = mybir.AluOpType

# ---------------------------------------------------------------- drain patch
# walrus allows at most ONE sync wait on CTRL/DMA instructions, but the Tile
# kernel-tail drain waits on every DMA sem lane used (up to 16). Split them.
def _patched_drain_and_barrier(self, tick_clock, wait_clock):
    drain_inst = self.nc.sync.drain()
    wait_clock.add_sem_waits(
        drain_inst.ins, ScopedClock({None: tick_clock.global_clock})
    )
    si = drain_inst.ins.sync_info
    waits = list(si.on_wait or []) if si is not None else []
    if len(waits) > 1:
        si.on_wait = waits[:1]
        for w in waits[1:]:
            extra = self.nc.sync.drain()
            esi = extra.ins.sync_info
            if esi is None:
                import bass_rust
                extra.ins.sync_info = bass_rust.SyncInfo(on_wait=[], on_update=[])
                esi = extra.ins.sync_info
            esi.on_wait = [w]
    self.nc.all_engine_barrier()
    assert self.sems is not None
    popped = self.nc._tile_sem_poison_stack.pop()
    assert popped is self._sem_poison
    self.nc.clear_and_free_semaphores(list(self.sems.allocated().values()))
    self.nc.all_engine_barrier()

tile.TileContext._drain_and_barrier = _patched_drain_and_barrier


_NC_CACHE = {}


def build(consts):
    ck = tuple(sorted(consts.items()))
    if ck in _NC_CACHE:
        return _NC_CACHE[ck]
    K = consts["K"]
    CPB = consts["CPB"]            # chunks per bank stream (= WPC*K)
    NCALLB = consts["NCALLB"]      # gather calls per bank
    NCHUNKS = consts["NCHUNKS"]
    NIDX = consts["NIDX"]          # 1024 idx per call
    S = consts["S"]                # idx cols per call (64)
    CW = BANKS * K                 # chunks per window (20)

    nc = bacc.Bacc("TRN2", target_bir_lowering=False, debug=False,
                   num_devices=CORES, num_swdge_queues=4)

    # ------------------------------------------------------------- tensors
    xT = nc.dram_tensor("xT", [P, N], F16, kind="ExternalInput")
    xTs = nc.dram_tensor("xTs", [P, M], F16, kind="ExternalInput")
    wcat1 = nc.dram_tensor("wcat1", [P, 132], F16, kind="ExternalInput")
    wcat2 = nc.dram_tensor("wcat2", [P, 132], F16, kind="ExternalInput")
    wad1 = nc.dram_tensor("wad1", [P, H], F16, kind="ExternalInput")
    wad2 = nc.dram_tensor("wad2", [P, H], F16, kind="ExternalInput")
    brow1 = nc.dram_tensor("brow1", [1, 132], F16, kind="ExternalInput")
    brow2 = nc.dram_tensor("brow2", [1, 132], F16, kind="ExternalInput")
    idxw = nc.dram_tensor("idxw", [P, BANKS * NCALLB * S], I16, kind="ExternalInput")
    dstrel = nc.dram_tensor("dstrel", [P, NCHUNKS], F32, kind="ExternalInput")
    out2 = nc.dram_tensor("out2", [M, C], F32, kind="ExternalOutput")

    table = [nc.dram_tensor(f"table{l}", [N, ROW], F16) for l in (1, 2)]
    h_shard = nc.dram_tensor("h_shard", [P, M], F16)
    h_full = nc.dram_tensor("h_full", [CORES, P, M], F16, addr_space="Shared")

    with tile.TileContext(nc) as tc:
        with (
            tc.tile_pool(name="const", bufs=1) as cpool,
            tc.tile_pool(name="resident", bufs=1) as rpool,
            tc.tile_pool(name="p0", bufs=4) as p0pool,
            tc.tile_pool(name="p0ps", bufs=2, space="PSUM") as p0ps,
            tc.tile_pool(name="gat", bufs=3) as gpool,
            tc.tile_pool(name="oh", bufs=2 * CW + 4) as ohpool,
            tc.tile_pool(name="cmp", bufs=3) as cmppool,
            tc.tile_pool(name="wps", bufs=2, space="PSUM") as wps,
            tc.tile_pool(name="trps", bufs=2, space="PSUM") as trps,
            tc.tile_pool(name="adps", bufs=2, space="PSUM") as adps,
            tc.tile_pool(name="p3", bufs=3) as p3pool,
        ):
            # ---------------- constants
            ident = cpool.tile([P, P], F16)
            make_identity(nc, ident[:])
            iota_i = cpool.tile([P, P], mybir.dt.int32)
            nc.gpsimd.iota(iota_i[:], pattern=[[1, P]], base=0, channel_multiplier=0)
            iota_row = cpool.tile([P, P], F32)
            nc.vector.tensor_copy(iota_row[:], iota_i[:])
            ones_row = cpool.tile([1, P], F16)
            nc.vector.memset(ones_row[:], 1.0)

            wc = []
            for l, t in ((0, wcat1), (1, wcat2)):
                w_t = cpool.tile([P, 132], F16, tag=f"wc{l}")
                nc.sync.dma_start(out=w_t[:], in_=t[:, :])
                wc.append(w_t)
            wad = []
            for l, t in ((0, wad1), (1, wad2)):
                w_t = cpool.tile([P, H], F16, tag=f"wad{l}")
                nc.sync.dma_start(out=w_t[:], in_=t[:, :])
                wad.append(w_t)
            br = []
            for l, t in ((0, brow1), (1, brow2)):
                w_t = cpool.tile([1, 132], F16, tag=f"br{l}")
                nc.sync.dma_start(out=w_t[:], in_=t[:, :])
                br.append(w_t)

            # resident edge-structure data (shared by both layers)
            idx_t = rpool.tile([P, BANKS * NCALLB * S], I16)
            nc.sync.dma_start(out=idx_t[:], in_=idxw[:, :])
            rel_t = rpool.tile([P, NCHUNKS], F32)
            nc.sync.dma_start(out=rel_t[:], in_=dstrel[:, :])

            # per-layer a_dst of the local shard: [128 dst-lane, WPC*H]
            adres0 = rpool.tile([P, WPC * H], F16, tag="ad0")
            adres1 = rpool.tile([P, WPC * H], F16, tag="ad1")
            adres = [adres0, adres1]
            nc.vector.memset(adres[0][:], 0.0)
            nc.vector.memset(adres[1][:], 0.0)

            # layer-1 a_dst: from the xT shard input
            for w in range(WPC):
                dsz = min(P, M - w * P)
                lt = p0pool.tile([P, P], F16, tag="adl")
                nc.sync.dma_start(out=lt[:, :dsz], in_=xTs[:, w * P:w * P + dsz])
                ps = adps.tile([P, H], F32, tag="adx")
                nc.tensor.matmul(ps[:dsz], lhsT=lt[:, :dsz], rhs=wad[0][:],
                                 start=True, stop=True)
                nc.vector.tensor_copy(adres[0][:dsz, w * H:(w + 1) * H], ps[:dsz])

            for L in range(2):
                # ======================================================= phase 0
                for t in range(NTILES):
                    tsz = min(P, N - t * P)
                    lt = p0pool.tile([P, P], F16, tag="p0l")
                    if L == 0:
                        nc.sync.dma_start(out=lt[:, :tsz], in_=xT[:, t * P:t * P + tsz])
                    else:
                        # h_full blocks of M columns each; a tile may span two
                        n0 = t * P
                        done = 0
                        while done < tsz:
                            blk = (n0 + done) // M
                            off = (n0 + done) % M
                            take = min(tsz - done, M - off)
                            nc.sync.dma_start(
                                out=lt[:, done:done + take],
                                in_=h_full[blk, :, off:off + take])
                            done += take
                    ps = p0ps.tile([P, 132], F32, tag="p0p")
                    nc.tensor.matmul(ps[:tsz], lhsT=lt[:, :tsz], rhs=wc[L][:],
                                     start=True, stop=False)
                    nc.tensor.matmul(ps[:tsz], lhsT=ones_row[:1, :tsz], rhs=br[L][:],
                                     start=False, stop=True)
                    st = p0pool.tile([P, 132], F16, tag="p0s")
                    nc.vector.tensor_copy(st[:tsz], ps[:tsz])
                    nc.sync.dma_start(out=table[L][t * P:t * P + tsz, 0:132],
                                      in_=st[:tsz])

                # ======================================================= edges
                nextcall = [0] * BANKS
                gtiles = [dict() for _ in range(BANKS)]
                for w in range(WPC):
                    dsz = min(P, M - w * P)
                    # issue gather calls covering this window's chunks
                    for b in range(BANKS):
                        while nextcall[b] * PAGE < min((w + 1) * K, CPB):
                            j = nextcall[b]
                            g = gpool.tile([P, PAGE, ROW], F16, tag=f"g{b}")
                            col0 = (b * NCALLB + j) * S
                            nc.gpsimd.dma_gather(
                                g[:], table[L][BANK_BASES[b]:BANK_BASES[b] + BANK_ROWS, :],
                                idx_t[:, col0:col0 + S], NIDX, NIDX, ROW,
                                queue_num=b)
                            gtiles[b][j] = g
                            if j - 2 in gtiles[b]:
                                del gtiles[b][j - 2]
                            nextcall[b] += 1

                    def chunk_rows(cw):
                        b, k = cw // K, cw % K
                        cglob = w * K + k
                        return gtiles[b][cglob // PAGE][:, cglob % PAGE, :]

                    # pass A: one-hots, a_dst expansion, e values
                    ew = cmppool.tile([P, CW * H], F32, tag="ew")
                    ohs = []
                    for cw in range(CW):
                        cid = (w * BANKS + (cw // K)) * K + (cw % K)
                        oh = ohpool.tile([P, P], F16, tag="oh")
                        nc.vector.tensor_scalar(
                            oh[:], iota_row[:], rel_t[:, cid:cid + 1], None,
                            op0=ALU.is_equal)
                        ohs.append(oh)
                        trp = trps.tile([P, P], F16, tag="tr")
                        nc.tensor.transpose(trp[:], oh[:], ident[:])
                        ohT = cmppool.tile([P, P], F16, tag="ohT")
                        nc.vector.tensor_copy(ohT[:], trp[:])
                        adp = adps.tile([P, H], F32, tag="adx")
                        nc.tensor.matmul(adp[:], lhsT=ohT[:],
                                         rhs=adres[L][:, w * H:(w + 1) * H],
                                         start=True, stop=True)
                        rows = chunk_rows(cw)
                        nc.vector.tensor_tensor(
                            ew[:, cw * H:(cw + 1) * H], adp[:], rows[:, F:F + H],
                            op=ALU.add)
                    # leaky-relu + exp over the whole window
                    lr = cmppool.tile([P, CW * H], F32, tag="lr")
                    nc.vector.tensor_scalar(lr[:], ew[:], NEG_SLOPE, None, op0=ALU.mult)
                    nc.vector.tensor_tensor(lr[:], lr[:], ew[:], op=ALU.max)
                    ex = cmppool.tile([P, CW * H], F16, tag="ex")
                    nc.scalar.activation(ex[:], lr[:], AF.Exp)

                    # pass B: rhs build + scatter matmuls into window PSUM
                    pw = wps.tile([P, 132], F32, tag="pw")
                    for cw in range(CW):
                        rows = chunk_rows(cw)
                        rhs = cmppool.tile([P, 132], F16, tag="rhs")
                        nc.vector.tensor_copy(rhs[:, 0:H], ex[:, cw * H:(cw + 1) * H])
                        nc.vector.tensor_tensor(
                            rhs[:, H:H + F].rearrange("p (h c) -> p h c", h=H),
                            rows[:, 0:F].rearrange("p (h c) -> p h c", h=H),
                            ex[:, cw * H:(cw + 1) * H, None].to_broadcast([P, H, C]),
                            op=ALU.mult)
                        nc.tensor.matmul(pw[:], lhsT=ohs[cw][:], rhs=rhs[:],
                                         start=(cw == 0), stop=(cw == CW - 1))

                    # ==================================================== phase 3
                    sp = p3pool.tile([P, H], F32, tag="s")
                    nc.vector.tensor_scalar(sp[:], pw[:, 0:H], EPS, None, op0=ALU.add)
                    r = p3pool.tile([P, H], F32, tag="r")
                    nc.vector.reciprocal(r[:], sp[:])
                    hw = p3pool.tile([P, F], F32, tag="hw")
                    nc.vector.tensor_tensor(
                        hw[:].rearrange("p (h c) -> p h c", h=H),
                        pw[:, H:H + F].rearrange("p (h c) -> p h c", h=H),
                        r[:, :, None].to_broadcast([P, H, C]),
                        op=ALU.mult)
                    if L == 0:
                        # elu, then transpose, stash hT and layer-2 a_dst
                        mn = p3pool.tile([P, F], F32, tag="mn")
                        nc.vector.tensor_scalar(mn[:], hw[:], 0.0, None, op0=ALU.min)
                        ek = p3pool.tile([P, F], F32, tag="ek")
                        nc.scalar.activation(ek[:], mn[:], AF.Exp)
                        he = p3pool.tile([P, F], F16, tag="he")
                        nc.vector.tensor_scalar(he[:], ek[:], -1.0, None, op0=ALU.add)
                        nc.vector.tensor_scalar(mn[:], hw[:], 0.0, None, op0=ALU.max)
                        nc.vector.tensor_tensor(he[:], he[:], mn[:], op=ALU.add)
                        trp = trps.tile([P, P], F16, tag="tr")
                        nc.tensor.transpose(trp[:], he[:], ident[:])
                        hT = p3pool.tile([P, P], F16, tag="hT")
                        nc.vector.tensor_copy(hT[:], trp[:])
                        adp = adps.tile([P, H], F32, tag="adx")
                        nc.tensor.matmul(adp[:dsz], lhsT=hT[:, :dsz], rhs=wad[1][:],
                                         start=True, stop=True)
                        nc.vector.tensor_copy(adres[1][:dsz, w * H:(w + 1) * H],
                                              adp[:dsz])
                        nc.sync.dma_start(out=h_shard[:, w * P:w * P + dsz],
                                          in_=hT[:, :dsz])
                    else:
                        om = p3pool.tile([P, C], F32, tag="om")
                        nc.vector.tensor_tensor(om[:], hw[:, 0:C], hw[:, C:2 * C],
                                                op=ALU.add)
                        nc.vector.tensor_tensor(om[:], om[:], hw[:, 2 * C:3 * C],
                                                op=ALU.add)
                        nc.vector.tensor_tensor(om[:], om[:], hw[:, 3 * C:4 * C],
                                                op=ALU.add)
                        nc.vector.tensor_scalar(om[:], om[:], 0.25, None, op0=ALU.mult)
                        nc.sync.dma_start(out=out2[w * P:w * P + dsz, :],
                                          in_=om[:dsz])

                if L == 0:
                    nc.gpsimd.collective_compute(
                        "AllGather", ALU.bypass,
                        replica_groups=[list(range(CORES))],
                        ins=[h_shard.ap()],
                        outs=[h_full.ap()],
                    )
    nc.compile()
    _NC_CACHE[ck] = nc
    return nc


def make_inmaps(inputs, perm, consts):
    x = np.asarray(inputs["x"], np.float32)
    cat1 = _cat_mats(np.asarray(inputs["W1"], np.float32),
                     np.asarray(inputs["att_src1"], np.float32),
                     np.asarray(inputs["att_dst1"], np.float32))   # [128, 136]
    cat2 = _cat_mats(np.asarray(inputs["W2"], np.float32),
                     np.asarray(inputs["att_src2"], np.float32),
                     np.asarray(inputs["att_dst2"], np.float32))
    xT_np = np.ascontiguousarray(x.T).astype(np.float16)
    b1 = np.asarray(inputs["b1"], np.float32)
    b2 = np.asarray(inputs["b2"], np.float32)
    brow1 = np.zeros((1, 132), np.float16); brow1[0, :F] = b1.astype(np.float16)
    brow2 = np.zeros((1, 132), np.float16); brow2[0, :F] = np.tile(b2, H).astype(np.float16)
    common = {
        "xT": xT_np,
        "wcat1": cat1[:, :132].astype(np.float16),
        "wcat2": cat2[:, :132].astype(np.float16),
        "wad1": cat1[:, 132:136].astype(np.float16),
        "wad2": cat2[:, 132:136].astype(np.float16),
        "brow1": brow1, "brow2": brow2,
    }
    maps = []
    for m in range(CORES):
        im = dict(common)
        im["xTs"] = np.ascontiguousarray(xT_np[:, m * M:(m + 1) * M])
        im["idxw"] = perm[m]["idxw"]
        im["dstrel"] = perm[m]["dstrel"]
        maps.append(im)
    return maps


def run_on_hw(inputs, perm, consts):
    nc = build(consts)
    maps = make_inmaps(inputs, perm, consts)
    res = run_bass_kernel_spmd(nc, maps, core_ids=list(range(CORES)))
    return np.concatenate([res.results[m]["out2"] for m in range(CORES)], axis=0)


def kernel(**inputs):
    perm, consts = _host_prep(np.asarray(inputs["edge_index"]))
    return run_on_hw(inputs, perm, consts)


